# revision 20
# baseline (speedup 1.0000x reference)
"""Trainium2 Bass kernel for nn_DistLayer (segment-mean pooling + fc + BatchNorm + ReLU).

Contract: kernel(**inputs) takes FULL unsharded numpy inputs and returns the
FULL [131072, 256] float32 output. Internally shards rows across 8 NeuronCores.

Math (reference):
    pooled_atom = segment_mean(x[:, :128], atom_idx)[atom_idx]
    pooled_ele  = segment_mean(x[:, 128:256], atom_idx)[ele_idx]
    h = concat([x_atom, pooled_atom, x_ele, pooled_ele, x_dist]) @ W1 + b1
    out = relu(batchnorm(h))                    (training-mode batch stats)

v3 structure (per core, h kept feature-major "h^T" [256, rows]):
  P1   : the N rows are cut into 64 global "units" of 2048 rows (8/core).
         Each unit OWNS a disjoint contiguous segment range (earliest unit
         containing a segment owns it; a unit's rows belonging to the previous
         unit's last segment are excluded from its one-hot, and a 5th "halo"
         tile -- the next unit's first 512 rows -- completes the last owned
         segment). One-hot matmuls produce COMPLETE global segment sums per
         owned lane, scaled by 1/count and flushed (f16) DENSELY into the
         core's slab: no scatter, no table zeroing, no cast pass.
  AG   : AllGather the 8 per-core slabs -> gslab [8, ngrp*SLOT, 256] f16.
         Gather indices are host-remapped to (core, unit, lane) slots. Issued
         early; overlaps phase A.
  A    : h_x^T = Wx^T x^T per block, flushed to persistent SBUF hsb.
  B    : one merged dma_gather per block pulls both pooled_atom and pooled_ele
         rows from gslab; two f16 matmuls per 128-feature chunk; DVE
         tensor_tensor_reduce adds PSUM into hsb and emits sum(h) per block;
         an Act Square pass emits sum(h^2).
  stats: AllGather per-core [128,4] sums, reduce locally, build scale/bias.
  P3   : fused scale+bias+relu pass streamed to DRAM out.
"""

import os
from contextlib import ExitStack

import numpy as np

import concourse.bass as bass
import concourse.tile as tile
from concourse import bacc, mybir
from concourse.bass_utils import run_bass_kernel_spmd

LAST_NC = None  # most recent built program (for cost-model timing in test.py)
SAFE = set(filter(None, os.environ.get("K_SAFE", "").split(","))) | {"ttr", "mg"}

F32 = mybir.dt.float32
F32R = mybir.dt.float32r
F16 = mybir.dt.float16
I16 = mybir.dt.int16
I32 = mybir.dt.int32

N_AE = 128
NUM_SEG = 4096
EPS = 1e-5
D_IN = 384            # x feature dim
D_OUT = 256           # output feature dim
BLK = 512             # rows per block
TPB = BLK // 128      # row-tiles per block
GSZ = 4               # blocks per unit (2048 rows)


def _wrap_idx16(idx):
    """dma_gather index layout: idx i at [i%16, i//16], replicated to 128 partitions."""
    n = idx.shape[0]
    w = idx.reshape(n // 16, 16).T.astype(np.int16)   # [16, n/16]
    return np.tile(w, (8, 1))                          # [128, n/16]


def build_program(n_cores, rpc, slot):
    """Build the (core-uniform) bass program. rpc = rows per core."""
    nblk = rpc // BLK
    ngrp = nblk // GSZ
    W = ngrp * slot       # slab rows per core
    nc = bacc.Bacc("TRN2", target_bir_lowering=False, debug=False,
                   num_devices=n_cores)

    # ---- I/O tensors (per-core) ----
    d_xt = nc.dram_tensor("xt", [nblk, 128, 3 * BLK], F16, kind="ExternalInput").ap()
    d_xae = nc.dram_tensor("xae", [nblk + 1, 128, TPB * 2 * N_AE], F16, kind="ExternalInput").ap()
    d_oh = nc.dram_tensor("oh", [ngrp, 128, (GSZ + 1) * TPB * slot], F16, kind="ExternalInput").ap()
    d_scl = nc.dram_tensor("scl", [slot, ngrp], F32, kind="ExternalInput").ap()
    d_gidx = nc.dram_tensor("gidx", [nblk, 128, 2 * (BLK // 16)], I16, kind="ExternalInput").ap()
    if "mg" in SAFE:
        d_gidxa = nc.dram_tensor("gidxa", [nblk, 128, BLK // 16], I16, kind="ExternalInput").ap()
        d_gidxe = nc.dram_tensor("gidxe", [nblk, 128, BLK // 16], I16, kind="ExternalInput").ap()
    d_wx = nc.dram_tensor("wx", [D_IN, D_OUT], F16, kind="ExternalInput").ap()
    d_wpa = nc.dram_tensor("wpa", [N_AE, D_OUT], F16, kind="ExternalInput").ap()
    d_wpe = nc.dram_tensor("wpe", [N_AE, D_OUT], F16, kind="ExternalInput").ap()
    d_gb = nc.dram_tensor("gb", [128, 4], F32, kind="ExternalInput").ap()

    d_out = nc.dram_tensor("out", [D_OUT, rpc], F32, kind="ExternalOutput").ap()
    if "dbg" in SAFE:
        d_dbg1 = nc.dram_tensor("dbg1", [n_cores * W * 2, 128], F16, kind="ExternalOutput").ap()
        d_dbg2 = nc.dram_tensor("dbg2", [128, rpc], F32, kind="ExternalOutput").ap()
        d_dbg3 = nc.dram_tensor("dbg3", [128, 12], F32, kind="ExternalOutput").ap()

    groups = [list(range(n_cores))]

    with tile.TileContext(nc) as tc, ExitStack() as ctx:
        const = ctx.enter_context(tc.tile_pool(name="const", bufs=1))
        store = ctx.enter_context(tc.tile_pool(name="store", bufs=1))
        strm = ctx.enter_context(tc.tile_pool(name="strm", bufs=3))
        ps = ctx.enter_context(tc.tile_pool(name="ps", bufs=2, space="PSUM"))
        dram = ctx.enter_context(tc.tile_pool(name="dram", bufs=1, space="DRAM"))

        # internal DRAM
        pslab = dram.tile([W, D_OUT], F16)                # AG input slab
        gslab = dram.tile([n_cores, W, D_OUT], F16, addr_space="Shared")
        statin = dram.tile([128, 4], F32)
        statout = dram.tile([128, 4], F32, addr_space="Shared")

        # ---- constants in SBUF ----
        wxr = const.tile([128, 3 * D_OUT], F16)
        nc.gpsimd.dma_start(wxr[:].rearrange("p (c f) -> p c f", c=3),
                            d_wx.rearrange("(c p) f -> p c f", p=128))
        wpa = const.tile([128, D_OUT], F16)
        nc.scalar.dma_start(wpa[:], d_wpa[:])
        wpe = const.tile([128, D_OUT], F16)
        nc.scalar.dma_start(wpe[:], d_wpe[:])
        scl = const.tile([slot, ngrp], F32)
        nc.scalar.dma_start(scl[:], d_scl[:])
        gb = const.tile([128, 4], F32)
        nc.scalar.dma_start(gb[:], d_gb[:])
        gsb = const.tile([128, nblk * 2 * (BLK // 16)], I16)
        nc.scalar.dma_start(gsb[:].rearrange("p (b w) -> p b w", b=nblk),
                            d_gidx.rearrange("b p w -> p b w"))
        if "mg" in SAFE:
            gsba = const.tile([128, nblk * (BLK // 16)], I16)
            nc.scalar.dma_start(gsba[:].rearrange("p (b w) -> p b w", b=nblk),
                                d_gidxa.rearrange("b p w -> p b w"))
            gsbe = const.tile([128, nblk * (BLK // 16)], I16)
            nc.scalar.dma_start(gsbe[:].rearrange("p (b w) -> p b w", b=nblk),
                                d_gidxe.rearrange("b p w -> p b w"))

        # persistent h^T store: 2 chunks of [128, rpc]
        hsb = [store.tile([128, rpc], F32, name=f"hsb{m}", tag=f"hsb{m}")
               for m in range(2)]
        sums = store.tile([128, 6 * nblk], F32)   # [shq0|shq1|shh0|shh1|sa0|sa1]

        # ---- P1: per-unit one-hot matmuls -> dense f16 slab flush ----
        XW = TPB * 2 * N_AE
        xtiles = []

        def load_xae(b):
            t = strm.tile([128, XW], F16, name="xae", tag="xae", bufs=9)
            nc.sync.dma_start(t[:], d_xae[b])
            xtiles.append(t)

        for b in range(GSZ):
            load_xae(b)
        for g in range(ngrp):
            for b in range(GSZ * (g + 1), min(GSZ * (g + 2), nblk + 1)):
                load_xae(b)
            ohx = strm.tile([128, (GSZ + 1) * TPB * slot], F16,
                            name="ohx", tag="ohx", bufs=2)
            nc.sync.dma_start(ohx[:], d_oh[g])
            seg = ps.tile([slot, D_OUT], F32, name="seg", tag="seg")
            for j in range(GSZ + 1):
                xt_b = xtiles[GSZ * g + j]
                for t in range(TPB):
                    nc.tensor.matmul(seg[:],
                                     ohx[:, (TPB * j + t) * slot:
                                         (TPB * j + t + 1) * slot],
                                     xt_b[:, 2 * N_AE * t:2 * N_AE * (t + 1)],
                                     start=(j == 0 and t == 0),
                                     stop=(j == GSZ and t == TPB - 1))
            ssb = strm.tile([slot, D_OUT], F16, name="ssb", tag="ssb", bufs=2)
            # scale by 1/global_count while flushing PSUM -> SBUF (to f16)
            nc.scalar.activation(ssb[:], seg[:],
                                 mybir.ActivationFunctionType.Identity,
                                 bias=0.0, scale=scl[:, g:g + 1])
            nc.scalar.dma_start(pslab[slot * g:slot * (g + 1)], ssb[:])

        # ---- AllGather the slabs ----
        nc.gpsimd.collective_compute(
            "AllGather", mybir.AluOpType.bypass, replica_groups=groups,
            ins=[pslab.opt()], outs=[gslab.opt()])

        # ---- phase A: h_x^T = Wx^T x^T per block -> hsb ----
        # hold the x loads out of the scheduler's P1 window so the slab
        # AllGather (which gates phase B) is issued as early as possible
        for b in range(nblk):
            xtr = strm.tile([128, 3 * BLK], F16, name="xtr", tag="xtr")
            with tc.tile_wait_until(0.030, enable="wait" not in SAFE):
                nc.sync.dma_start(xtr[:], d_xt[b])
            for m in range(2):
                hp = ps.tile([128, BLK], F32, name=f"hp{m}", tag=f"hp{m}")
                for k in range(3):
                    nc.tensor.matmul(hp[:],
                                     wxr[:, D_OUT * k + 128 * m:
                                         D_OUT * k + 128 * (m + 1)],
                                     xtr[:, BLK * k:BLK * (k + 1)],
                                     start=(k == 0), stop=(k == 2))
                if "ttr" in SAFE:
                    nc.scalar.activation(
                        hsb[m][:, BLK * b:BLK * (b + 1)], hp[:],
                        mybir.ActivationFunctionType.Copy,
                        accum_out=sums[:, 4 * nblk + nblk * m + b:
                                       4 * nblk + nblk * m + b + 1])
                else:
                    nc.scalar.copy(hsb[m][:, BLK * b:BLK * (b + 1)], hp[:])

        # ---- phase B: merged gather + pooled matmuls, add into hsb ----
        gview = gslab[:].rearrange("s w (t f) -> (s w t) f", t=2)
        gview2 = gslab[:].rearrange("s w f -> (s w) f")
        for b in range(nblk):
            gat = strm.tile([128, 2 * BLK], F16, name="gat", tag="gat", bufs=4)
            if "mg" in SAFE:
                nc.gpsimd.dma_gather(
                    out_ap=gat[:, 0:BLK].rearrange("p (a n) -> p a n", a=1),
                    in_ap=gview2[:, 0:N_AE],
                    idxs_ap=gsba[:, (BLK // 16) * b:(BLK // 16) * (b + 1)],
                    num_idxs=BLK, num_idxs_reg=BLK,
                    elem_size=N_AE, elem_step=D_OUT, transpose=True)
                nc.gpsimd.dma_gather(
                    out_ap=gat[:, BLK:2 * BLK].rearrange("p (a n) -> p a n", a=1),
                    in_ap=gview2[:, N_AE:2 * N_AE],
                    idxs_ap=gsbe[:, (BLK // 16) * b:(BLK // 16) * (b + 1)],
                    num_idxs=BLK, num_idxs_reg=BLK,
                    elem_size=N_AE, elem_step=D_OUT, transpose=True)
            else:
                nc.gpsimd.dma_gather(
                    out_ap=gat[:].rearrange("p (a n) -> p a n", a=1),
                    in_ap=gview,
                    idxs_ap=gsb[:, 2 * (BLK // 16) * b:2 * (BLK // 16) * (b + 1)],
                    num_idxs=2 * BLK, num_idxs_reg=2 * BLK,
                    elem_size=N_AE, elem_step=N_AE, transpose=True)
            for m in range(2):
                hq = ps.tile([128, BLK], F32, name=f"hp{m}", tag=f"hp{m}")
                nc.tensor.matmul(hq[:], wpa[:, 128 * m:128 * (m + 1)],
                                 gat[:, 0:BLK], start=True, stop=False)
                nc.tensor.matmul(hq[:], wpe[:, 128 * m:128 * (m + 1)],
                                 gat[:, BLK:2 * BLK], start=False, stop=True)
                # hsb += pooled part; accumulate sum(h) on the fly
                if "ttr" in SAFE:
                    hqs = strm.tile([128, BLK], F32, name="hqs", tag="hqs", bufs=3)
                    nc.scalar.activation(
                        hqs[:], hq[:],
                        mybir.ActivationFunctionType.Copy,
                        accum_out=sums[:, nblk * m + b:nblk * m + b + 1])
                    nc.vector.tensor_tensor(
                        out=hsb[m][:, BLK * b:BLK * (b + 1)],
                        in0=hsb[m][:, BLK * b:BLK * (b + 1)],
                        in1=hqs[:], op=mybir.AluOpType.add)
                else:
                    nc.vector.scalar_tensor_tensor(
                        out=hsb[m][:, BLK * b:BLK * (b + 1)],
                        in0=hsb[m][:, BLK * b:BLK * (b + 1)],
                        scalar=0.0, in1=hq[:],
                        op0=mybir.AluOpType.add, op1=mybir.AluOpType.add,
                        accum_out=sums[:, nblk * m + b:nblk * m + b + 1])
                dump = strm.tile([128, BLK], F32, name="dump", tag="dump", bufs=1)
                nc.scalar.activation(
                    dump[:], hsb[m][:, BLK * b:BLK * (b + 1)],
                    mybir.ActivationFunctionType.Square,
                    accum_out=sums[:, 2 * nblk + nblk * m + b:
                                   2 * nblk + nblk * m + b + 1])

        # ---- BN stats: reduce blocks, AllGather cores, build scale/bias ----
        s4raw = const.tile([128, 6], F32)
        for j in range(6):
            nc.vector.reduce_sum(s4raw[:, j:j + 1],
                                 sums[:, nblk * j:nblk * (j + 1)],
                                 axis=mybir.AxisListType.X)
        s4 = const.tile([128, 4], F32)
        nc.vector.tensor_tensor(out=s4[:, 0:2], in0=s4raw[:, 0:2],
                                in1=s4raw[:, 4:6], op=mybir.AluOpType.add)
        nc.vector.tensor_scalar_add(s4[:, 2:4], s4raw[:, 2:4], 0.0)
        nc.sync.dma_start(statin[:], s4[:])
        nc.gpsimd.collective_compute(
            "AllReduce", mybir.AluOpType.add, replica_groups=groups,
            ins=[statin.opt()], outs=[statout.opt()])
        s4g = const.tile([128, 4], F32)
        nc.sync.dma_start(s4g[:], statout[:])

        n_total = float(n_cores * rpc)
        mu = const.tile([128, 2], F32)
        nc.scalar.mul(mu[:], s4g[:, 0:2], 1.0 / n_total)
        ex2 = const.tile([128, 2], F32)
        nc.scalar.mul(ex2[:], s4g[:, 2:4], 1.0 / n_total)
        mu2 = const.tile([128, 2], F32)
        nc.vector.tensor_tensor(out=mu2[:], in0=mu[:], in1=mu[:],
                                op=mybir.AluOpType.mult)
        nmu2 = const.tile([128, 2], F32)
        nc.scalar.mul(nmu2[:], mu2[:], -1.0)
        var = const.tile([128, 2], F32)
        nc.vector.tensor_tensor(out=var[:], in0=ex2[:], in1=nmu2[:],
                                op=mybir.AluOpType.add)
        vare = const.tile([128, 2], F32)
        nc.vector.tensor_scalar_add(vare[:], var[:], EPS)
        std = const.tile([128, 2], F32)
        nc.scalar.activation(std[:], vare[:], mybir.ActivationFunctionType.Sqrt,
                             bias=0.0)
        rstd = const.tile([128, 2], F32)
        nc.vector.reciprocal(rstd[:], std[:])
        # a = gamma * rstd ; baff = beta - mu * a
        a_t = const.tile([128, 2], F32)
        nc.vector.tensor_tensor(out=a_t[:], in0=gb[:, 0:2], in1=rstd[:],
                                op=mybir.AluOpType.mult)
        mua = const.tile([128, 2], F32)
        nc.vector.tensor_tensor(out=mua[:], in0=mu[:], in1=a_t[:],
                                op=mybir.AluOpType.mult)
        nmua = const.tile([128, 2], F32)
        nc.scalar.mul(nmua[:], mua[:], -1.0)
        baff = const.tile([128, 2], F32)
        nc.vector.tensor_tensor(out=baff[:], in0=gb[:, 2:4], in1=nmua[:],
                                op=mybir.AluOpType.add)

        if "dbg" in SAFE:
            nzz = n_cores * W * 2 // 128
            dbt = const.tile([128, nzz * 128], F16, name="dbt")
            nc.sync.dma_start(dbt[:].rearrange("p (c f) -> p c f", c=nzz),
                              gslab[:].rearrange("s w (t f) -> (s w t) f", t=2)
                              .rearrange("(c p) f -> p c f", p=128))
            nc.sync.dma_start(d_dbg1.rearrange("(c p) f -> p c f", p=128),
                              dbt[:].rearrange("p (c f) -> p c f", c=nzz))
            nc.sync.dma_start(d_dbg2, hsb[0][:])
            db3 = const.tile([128, 12], F32, name="db3")
            nc.vector.tensor_tensor(out=db3[:, 0:4], in0=s4g[:], in1=s4g[:],
                                    op=mybir.AluOpType.bypass)
            nc.vector.tensor_tensor(out=db3[:, 4:6], in0=mu[:], in1=mu[:],
                                    op=mybir.AluOpType.bypass)
            nc.vector.tensor_tensor(out=db3[:, 6:8], in0=var[:], in1=var[:],
                                    op=mybir.AluOpType.bypass)
            nc.vector.tensor_tensor(out=db3[:, 8:10], in0=a_t[:], in1=a_t[:],
                                    op=mybir.AluOpType.bypass)
            nc.vector.tensor_tensor(out=db3[:, 10:12], in0=s4[:, 0:2],
                                    in1=s4[:, 0:2], op=mybir.AluOpType.bypass)
            nc.sync.dma_start(d_dbg3, db3[:])

        # ---- P3: out = relu(h * a + b), streamed out ----
        d_out_v = d_out.rearrange("(c p) n -> p c n", p=128)
        for b in range(nblk):
            osb = strm.tile([128, 2 * BLK], F32, name="osb", tag="osb", bufs=4)
            for m in range(2):
                nc.scalar.activation(osb[:, BLK * m:BLK * (m + 1)],
                                     hsb[m][:, BLK * b:BLK * (b + 1)],
                                     mybir.ActivationFunctionType.Relu,
                                     scale=a_t[:, m:m + 1], bias=baff[:, m:m + 1])
            eng = nc.sync if (b % 2 == 0 or "gpst" in SAFE) else nc.gpsimd
            eng.dma_start(
                d_out_v[:, :, BLK * b:BLK * (b + 1)],
                osb[:].rearrange("p (c n) -> p c n", c=2))

    nc.compile()
    return nc


def prep_core_inputs(x, atom_idx, r0, r1, slot, gstart, glast, inv_cnt, n):
    """Host-side shard prep for one core (unit metadata is global).
    gstart/glast: per-global-unit owned-range arrays."""
    rpc = r1 - r0
    nblk = rpc // BLK
    ngrp = nblk // GSZ
    u0 = r0 // (BLK * GSZ)          # first global unit of this core
    xs = x[r0:r1]

    # pre-tiled transposed x: [block, partition(=feat%128), chunk*512+row]
    xt = np.ascontiguousarray(
        xs.reshape(nblk, BLK, 3, 128).transpose(0, 3, 2, 1)
    ).reshape(nblk, 128, 3 * BLK)

    # x_ae row-major f16 tiles: nblk own blocks + 1 halo block (next core)
    XW = TPB * 2 * N_AE
    xae = np.zeros((nblk + 1, 128, XW), dtype=np.float16)
    xae[:nblk] = xs[:, :2 * N_AE].astype(np.float16).reshape(
        nblk, TPB, 128, 2 * N_AE).transpose(0, 2, 1, 3).reshape(nblk, 128, XW)
    if r1 < n:
        xae[nblk] = x[r1:r1 + BLK, :2 * N_AE].astype(np.float16).reshape(
            TPB, 128, 2 * N_AE).transpose(1, 0, 2).reshape(128, XW)

    # one-hot tiles per unit: [ngrp, 128, (GSZ+1)*TPB*slot]
    oh = np.zeros((ngrp, 128, (GSZ + 1) * TPB * slot), dtype=np.float16)
    scl = np.zeros((slot, ngrp), dtype=np.float32)
    lanes = np.arange(slot)
    for g in range(ngrp):
        u = u0 + g
        gs, gl = gstart[u], glast[u]
        scl[:, g] = inv_cnt[np.minimum(gs + lanes, NUM_SEG - 1)]
        ubase = r0 + BLK * GSZ * g
        for j in range(GSZ + 1):
            if j < GSZ:
                rows = atom_idx[ubase + BLK * j:ubase + BLK * (j + 1)]
                sel = rows >= gs                   # exclude prev unit's seg
            else:
                rr0 = ubase + BLK * GSZ
                rows = atom_idx[rr0:rr0 + BLK]
                if len(rows) == 0:
                    continue
                sel = rows == gl                   # halo: only our last seg
            lane = (rows - gs).astype(np.int64)
            for t in range(TPB):
                s = sel[128 * t:128 * (t + 1)]
                ln = lane[128 * t:128 * (t + 1)]
                p = np.flatnonzero(np.asarray(s))
                oh[g, p, (TPB * j + t) * slot + ln[p]] = 1.0

    return {"xt": xt.astype(np.float16), "xae": xae, "oh": oh, "scl": scl}


def prep(x, atom_idx, ele_idx, W1, b1, gamma, beta, n_cores=8, build=True):
    x = np.asarray(x, dtype=np.float32)
    atom_idx = np.asarray(atom_idx).astype(np.int64)
    ele_idx = np.asarray(ele_idx).astype(np.int64)
    W1 = np.asarray(W1, dtype=np.float32)
    gamma = np.asarray(gamma, dtype=np.float32)
    beta = np.asarray(beta, dtype=np.float32)

    n = x.shape[0]
    assert n % n_cores == 0
    rpc = n // n_cores
    assert rpc % (BLK * GSZ) == 0
    nblk = rpc // BLK
    ngrp = nblk // GSZ
    nunit = n_cores * ngrp
    urows = BLK * GSZ
    assert np.all(np.diff(atom_idx) >= 0), "atom_idx must be sorted"

    counts = np.bincount(atom_idx, minlength=NUM_SEG).astype(np.int64)
    inv_cnt = (1.0 / np.maximum(counts, 1)).astype(np.float32)

    # global unit ownership: unit u owns segs [gstart[u], glast[u]]
    ufirst = atom_idx[np.arange(nunit) * urows]            # seg of first row
    ulast = atom_idx[np.arange(1, nunit + 1) * urows - 1]  # seg of last row
    gstart = np.empty(nunit, dtype=np.int64)
    gstart[0] = 0
    for u in range(1, nunit):
        gstart[u] = max(ufirst[u], ulast[u - 1] + 1)
    spans = ulast - gstart + 1
    assert spans.min() >= 1, "a unit owns no segment"
    slot = int(((spans.max() + 2) + 7) // 8 * 8)
    assert slot <= 128, f"unit owned span too large: {spans.max()}"
    # halo containment: a unit's last seg may spill only into the next
    # unit's first 512 rows
    for u in range(nunit - 1):
        r = urows * (u + 1) + BLK
        if r < n:
            assert atom_idx[r] > ulast[u], "segment spans past halo window"

    # global seg -> merged slab row (2*row for atom half, 2*row+1 for ele)
    W = ngrp * slot
    owner = np.full(NUM_SEG, -1, dtype=np.int64)
    for u in range(nunit - 1, -1, -1):
        owner[gstart[u]:ulast[u] + 1] = u
    slabrow = np.full(NUM_SEG, slot - 1, dtype=np.int64)   # empty segs -> zero row
    m = owner >= 0
    su = owner[m]
    slabrow[m] = (su // ngrp) * W + (su % ngrp) * slot + \
        (np.arange(NUM_SEG)[m] - gstart[su])
    arow = 2 * slabrow[atom_idx]
    erow = 2 * slabrow[ele_idx] + 1

    in_maps = []
    for c in range(n_cores):
        r0 = rpc * c
        im = prep_core_inputs(x, atom_idx, r0, r0 + rpc, slot, gstart, glast=ulast,
                              inv_cnt=inv_cnt, n=n)
        gidx = np.zeros((nblk, 128, 2 * (BLK // 16)), dtype=np.int16)
        for b in range(nblk):
            merged = np.concatenate([arow[r0 + BLK * b:r0 + BLK * (b + 1)],
                                     erow[r0 + BLK * b:r0 + BLK * (b + 1)]])
            gidx[b] = _wrap_idx16(merged)
        im["gidx"] = gidx
        if "mg" in SAFE:
            ga = np.zeros((nblk, 128, BLK // 16), dtype=np.int16)
            ge = np.zeros((nblk, 128, BLK // 16), dtype=np.int16)
            for b in range(nblk):
                ga[b] = _wrap_idx16(slabrow[atom_idx[r0 + BLK * b:r0 + BLK * (b + 1)]])
                ge[b] = _wrap_idx16(slabrow[ele_idx[r0 + BLK * b:r0 + BLK * (b + 1)]])
            im["gidxa"] = ga
            im["gidxe"] = ge
        im["wx"] = np.ascontiguousarray(
            np.concatenate([W1[0:128], W1[256:384], W1[512:640]], axis=0)
        ).astype(np.float16)
        im["wpa"] = W1[128:256].astype(np.float16)
        im["wpe"] = W1[384:512].astype(np.float16)
        gbt = np.zeros((128, 4), dtype=np.float32)
        gbt[:, 0:2] = gamma.reshape(2, 128).T
        gbt[:, 2:4] = beta.reshape(2, 128).T
        im["gb"] = gbt
        in_maps.append(im)

    nc = build_program(n_cores, rpc, slot) if build else None
    global LAST_NC
    LAST_NC = nc
    return nc, in_maps


def run(x, atom_idx, ele_idx, W1, b1, gamma, beta, n_cores=8, runner=None):
    nc, in_maps = prep(x, atom_idx, ele_idx, W1, b1, gamma, beta, n_cores)
    if runner is None:
        res = run_bass_kernel_spmd(nc, in_maps, core_ids=list(range(n_cores)))
        outs = [res.results[c]["out"] for c in range(n_cores)]
    else:
        outs = runner(nc, in_maps)

    full = np.concatenate(outs, axis=1)          # [256, n]
    return np.ascontiguousarray(full.T)          # [n, 256]


def kernel(**inputs):
    return run(inputs["x"], inputs["atom_idx"], inputs["ele_idx"],
               inputs["W1"], inputs["b1"], inputs["gamma"], inputs["beta"])


# revision 21
# speedup vs baseline: 1.0906x; 1.0906x over previous
"""Trainium2 Bass kernel for nn_DistLayer (segment-mean pooling + fc + BatchNorm + ReLU).

Contract: kernel(**inputs) takes FULL unsharded numpy inputs and returns the
FULL [131072, 256] float32 output. Internally shards rows across 8 NeuronCores.

Math (reference):
    pooled_atom = segment_mean(x[:, :128], atom_idx)[atom_idx]
    pooled_ele  = segment_mean(x[:, 128:256], atom_idx)[ele_idx]
    h = concat([x_atom, pooled_atom, x_ele, pooled_ele, x_dist]) @ W1 + b1
    out = relu(batchnorm(h))                    (training-mode batch stats)

v3 structure (per core, h kept feature-major "h^T" [256, rows]):
  P1   : the N rows are cut into 64 global "units" of 2048 rows (8/core).
         Each unit OWNS a disjoint contiguous segment range (earliest unit
         containing a segment owns it; a unit's rows belonging to the previous
         unit's last segment are excluded from its one-hot, and a 5th "halo"
         tile -- the next unit's first 512 rows -- completes the last owned
         segment). One-hot matmuls produce COMPLETE global segment sums per
         owned lane, scaled by 1/count and flushed (f16) DENSELY into the
         core's slab: no scatter, no table zeroing, no cast pass.
  AG   : AllGather the 8 per-core slabs -> gslab [8, ngrp*SLOT, 256] f16.
         Gather indices are host-remapped to (core, unit, lane) slots. Issued
         early; overlaps phase A.
  A    : h_x^T = Wx^T x^T per block, flushed to persistent SBUF hsb.
  B    : one merged dma_gather per block pulls both pooled_atom and pooled_ele
         rows from gslab; two f16 matmuls per 128-feature chunk; DVE
         tensor_tensor_reduce adds PSUM into hsb and emits sum(h) per block;
         an Act Square pass emits sum(h^2).
  stats: AllGather per-core [128,4] sums, reduce locally, build scale/bias.
  P3   : fused scale+bias+relu pass streamed to DRAM out.
"""

import os
from contextlib import ExitStack

import numpy as np

import concourse.bass as bass
import concourse.tile as tile
from concourse import bacc, mybir
from concourse.bass_utils import run_bass_kernel_spmd

LAST_NC = None  # most recent built program (for cost-model timing in test.py)
SAFE = set(filter(None, os.environ.get("K_SAFE", "").split(","))) | {"mg"}

F32 = mybir.dt.float32
F32R = mybir.dt.float32r
F16 = mybir.dt.float16
I16 = mybir.dt.int16
I32 = mybir.dt.int32

N_AE = 128
NUM_SEG = 4096
EPS = 1e-5
D_IN = 384            # x feature dim
D_OUT = 256           # output feature dim
BLK = 512             # rows per block
TPB = BLK // 128      # row-tiles per block
GSZ = 4               # blocks per unit (2048 rows)


def _wrap_idx16(idx):
    """dma_gather index layout: idx i at [i%16, i//16], replicated to 128 partitions."""
    n = idx.shape[0]
    w = idx.reshape(n // 16, 16).T.astype(np.int16)   # [16, n/16]
    return np.tile(w, (8, 1))                          # [128, n/16]


def build_program(n_cores, rpc, slot):
    """Build the (core-uniform) bass program. rpc = rows per core."""
    nblk = rpc // BLK
    ngrp = nblk // GSZ
    W = ngrp * slot       # slab rows per core
    nc = bacc.Bacc("TRN2", target_bir_lowering=False, debug=False,
                   num_devices=n_cores)

    # ---- I/O tensors (per-core) ----
    d_xt = nc.dram_tensor("xt", [nblk, 128, 3 * BLK], F16, kind="ExternalInput").ap()
    d_xae = nc.dram_tensor("xae", [nblk + 1, 128, TPB * 2 * N_AE], F16, kind="ExternalInput").ap()
    d_oh = nc.dram_tensor("oh", [ngrp, 128, (GSZ + 1) * TPB * slot], F16, kind="ExternalInput").ap()
    d_scl = nc.dram_tensor("scl", [slot, ngrp], F32, kind="ExternalInput").ap()
    d_gidx = nc.dram_tensor("gidx", [nblk, 128, 2 * (BLK // 16)], I16, kind="ExternalInput").ap()
    if "mg" in SAFE:
        d_gidxa = nc.dram_tensor("gidxa", [nblk, 128, BLK // 16], I16, kind="ExternalInput").ap()
        d_gidxe = nc.dram_tensor("gidxe", [nblk, 128, BLK // 16], I16, kind="ExternalInput").ap()
    d_wx = nc.dram_tensor("wx", [D_IN, D_OUT], F16, kind="ExternalInput").ap()
    d_wpa = nc.dram_tensor("wpa", [N_AE, D_OUT], F16, kind="ExternalInput").ap()
    d_wpe = nc.dram_tensor("wpe", [N_AE, D_OUT], F16, kind="ExternalInput").ap()
    d_gb = nc.dram_tensor("gb", [128, 4], F32, kind="ExternalInput").ap()

    d_out = nc.dram_tensor("out", [D_OUT, rpc], F32, kind="ExternalOutput").ap()
    if "dbg" in SAFE:
        d_dbg1 = nc.dram_tensor("dbg1", [n_cores * W * 2, 128], F16, kind="ExternalOutput").ap()
        d_dbg2 = nc.dram_tensor("dbg2", [128, rpc], F32, kind="ExternalOutput").ap()
        d_dbg3 = nc.dram_tensor("dbg3", [128, 12], F32, kind="ExternalOutput").ap()

    groups = [list(range(n_cores))]

    with tile.TileContext(nc) as tc, ExitStack() as ctx:
        const = ctx.enter_context(tc.tile_pool(name="const", bufs=1))
        store = ctx.enter_context(tc.tile_pool(name="store", bufs=1))
        strm = ctx.enter_context(tc.tile_pool(name="strm", bufs=3))
        ps = ctx.enter_context(tc.tile_pool(name="ps", bufs=2, space="PSUM"))
        dram = ctx.enter_context(tc.tile_pool(name="dram", bufs=1, space="DRAM"))

        # internal DRAM
        pslab = dram.tile([W, D_OUT], F16)                # AG input slab
        gslab = dram.tile([n_cores, W, D_OUT], F16, addr_space="Shared")
        statin = dram.tile([128, 4], F32)
        statout = dram.tile([128, 4], F32, addr_space="Shared")

        # ---- constants in SBUF ----
        wxr = const.tile([128, 3 * D_OUT], F16)
        nc.gpsimd.dma_start(wxr[:].rearrange("p (c f) -> p c f", c=3),
                            d_wx.rearrange("(c p) f -> p c f", p=128))
        wpa = const.tile([128, D_OUT], F16)
        nc.scalar.dma_start(wpa[:], d_wpa[:])
        wpe = const.tile([128, D_OUT], F16)
        nc.scalar.dma_start(wpe[:], d_wpe[:])
        scl = const.tile([slot, ngrp], F32)
        nc.scalar.dma_start(scl[:], d_scl[:])
        gb = const.tile([128, 4], F32)
        nc.scalar.dma_start(gb[:], d_gb[:])
        gsb = const.tile([128, nblk * 2 * (BLK // 16)], I16)
        nc.scalar.dma_start(gsb[:].rearrange("p (b w) -> p b w", b=nblk),
                            d_gidx.rearrange("b p w -> p b w"))
        if "mg" in SAFE:
            gsba = const.tile([128, nblk * (BLK // 16)], I16)
            nc.scalar.dma_start(gsba[:].rearrange("p (b w) -> p b w", b=nblk),
                                d_gidxa.rearrange("b p w -> p b w"))
            gsbe = const.tile([128, nblk * (BLK // 16)], I16)
            nc.scalar.dma_start(gsbe[:].rearrange("p (b w) -> p b w", b=nblk),
                                d_gidxe.rearrange("b p w -> p b w"))

        # persistent h^T store: 2 chunks of [128, rpc]
        hsb = [store.tile([128, rpc], F32, name=f"hsb{m}", tag=f"hsb{m}")
               for m in range(2)]
        sums = store.tile([128, 6 * nblk], F32)   # [shq0|shq1|shh0|shh1|sa0|sa1]

        # ---- P1: per-unit one-hot matmuls -> dense f16 slab flush ----
        XW = TPB * 2 * N_AE
        xtiles = []

        def load_xae(b):
            t = strm.tile([128, XW], F16, name="xae", tag="xae", bufs=9)
            nc.sync.dma_start(t[:], d_xae[b])
            xtiles.append(t)

        for b in range(GSZ):
            load_xae(b)
        for g in range(ngrp):
            for b in range(GSZ * (g + 1), min(GSZ * (g + 2), nblk + 1)):
                load_xae(b)
            ohx = strm.tile([128, (GSZ + 1) * TPB * slot], F16,
                            name="ohx", tag="ohx", bufs=2)
            nc.sync.dma_start(ohx[:], d_oh[g])
            seg = ps.tile([slot, D_OUT], F32, name="seg", tag="seg")
            for j in range(GSZ + 1):
                xt_b = xtiles[GSZ * g + j]
                for t in range(TPB):
                    nc.tensor.matmul(seg[:],
                                     ohx[:, (TPB * j + t) * slot:
                                         (TPB * j + t + 1) * slot],
                                     xt_b[:, 2 * N_AE * t:2 * N_AE * (t + 1)],
                                     start=(j == 0 and t == 0),
                                     stop=(j == GSZ and t == TPB - 1))
            ssb = strm.tile([slot, D_OUT], F16, name="ssb", tag="ssb", bufs=2)
            # scale by 1/global_count while flushing PSUM -> SBUF (to f16)
            nc.scalar.activation(ssb[:], seg[:],
                                 mybir.ActivationFunctionType.Identity,
                                 bias=0.0, scale=scl[:, g:g + 1])
            nc.scalar.dma_start(pslab[slot * g:slot * (g + 1)], ssb[:])

        # ---- AllGather the slabs ----
        nc.gpsimd.collective_compute(
            "AllGather", mybir.AluOpType.bypass, replica_groups=groups,
            ins=[pslab.opt()], outs=[gslab.opt()])

        # ---- phase A: h_x^T = Wx^T x^T per block -> hsb ----
        # hold the x loads out of the scheduler's P1 window so the slab
        # AllGather (which gates phase B) is issued as early as possible
        for b in range(nblk):
            xtr = strm.tile([128, 3 * BLK], F16, name="xtr", tag="xtr")
            with tc.tile_wait_until(0.030, enable="wait" not in SAFE):
                nc.sync.dma_start(xtr[:], d_xt[b])
            for m in range(2):
                hp = ps.tile([128, BLK], F32, name=f"hp{m}", tag=f"hp{m}")
                for k in range(3):
                    nc.tensor.matmul(hp[:],
                                     wxr[:, D_OUT * k + 128 * m:
                                         D_OUT * k + 128 * (m + 1)],
                                     xtr[:, BLK * k:BLK * (k + 1)],
                                     start=(k == 0), stop=(k == 2))
                if "ttr" in SAFE:
                    nc.scalar.activation(
                        hsb[m][:, BLK * b:BLK * (b + 1)], hp[:],
                        mybir.ActivationFunctionType.Copy,
                        accum_out=sums[:, 4 * nblk + nblk * m + b:
                                       4 * nblk + nblk * m + b + 1])
                else:
                    nc.scalar.copy(hsb[m][:, BLK * b:BLK * (b + 1)], hp[:])

        # ---- phase B: merged gather + pooled matmuls, add into hsb ----
        gview = gslab[:].rearrange("s w (t f) -> (s w t) f", t=2)
        gview2 = gslab[:].rearrange("s w f -> (s w) f")
        for b in range(nblk):
            gat = strm.tile([128, 2 * BLK], F16, name="gat", tag="gat", bufs=4)
            if "mg" in SAFE:
                nc.gpsimd.dma_gather(
                    out_ap=gat[:, 0:BLK].rearrange("p (a n) -> p a n", a=1),
                    in_ap=gview2[:, 0:N_AE],
                    idxs_ap=gsba[:, (BLK // 16) * b:(BLK // 16) * (b + 1)],
                    num_idxs=BLK, num_idxs_reg=BLK,
                    elem_size=N_AE, elem_step=D_OUT, transpose=True)
                nc.gpsimd.dma_gather(
                    out_ap=gat[:, BLK:2 * BLK].rearrange("p (a n) -> p a n", a=1),
                    in_ap=gview2[:, N_AE:2 * N_AE],
                    idxs_ap=gsbe[:, (BLK // 16) * b:(BLK // 16) * (b + 1)],
                    num_idxs=BLK, num_idxs_reg=BLK,
                    elem_size=N_AE, elem_step=D_OUT, transpose=True)
            else:
                nc.gpsimd.dma_gather(
                    out_ap=gat[:].rearrange("p (a n) -> p a n", a=1),
                    in_ap=gview,
                    idxs_ap=gsb[:, 2 * (BLK // 16) * b:2 * (BLK // 16) * (b + 1)],
                    num_idxs=2 * BLK, num_idxs_reg=2 * BLK,
                    elem_size=N_AE, elem_step=N_AE, transpose=True)
            for m in range(2):
                hq = ps.tile([128, BLK], F32, name=f"hp{m}", tag=f"hp{m}")
                nc.tensor.matmul(hq[:], wpa[:, 128 * m:128 * (m + 1)],
                                 gat[:, 0:BLK], start=True, stop=False)
                nc.tensor.matmul(hq[:], wpe[:, 128 * m:128 * (m + 1)],
                                 gat[:, BLK:2 * BLK], start=False, stop=True)
                # hsb += pooled part; accumulate sum(h) on the fly
                if "ttr" in SAFE:
                    hqs = strm.tile([128, BLK], F32, name="hqs", tag="hqs", bufs=3)
                    nc.scalar.activation(
                        hqs[:], hq[:],
                        mybir.ActivationFunctionType.Copy,
                        accum_out=sums[:, nblk * m + b:nblk * m + b + 1])
                    nc.vector.tensor_tensor(
                        out=hsb[m][:, BLK * b:BLK * (b + 1)],
                        in0=hsb[m][:, BLK * b:BLK * (b + 1)],
                        in1=hqs[:], op=mybir.AluOpType.add)
                else:
                    nc.vector.scalar_tensor_tensor(
                        out=hsb[m][:, BLK * b:BLK * (b + 1)],
                        in0=hsb[m][:, BLK * b:BLK * (b + 1)],
                        scalar=0.0, in1=hq[:],
                        op0=mybir.AluOpType.add, op1=mybir.AluOpType.add,
                        accum_out=sums[:, nblk * m + b:nblk * m + b + 1])
                dump = strm.tile([128, BLK], F32, name="dump", tag="dump", bufs=1)
                nc.scalar.activation(
                    dump[:], hsb[m][:, BLK * b:BLK * (b + 1)],
                    mybir.ActivationFunctionType.Square,
                    accum_out=sums[:, 2 * nblk + nblk * m + b:
                                   2 * nblk + nblk * m + b + 1])

        # ---- BN stats: reduce blocks, AllGather cores, build scale/bias ----
        nrange = 6 if "ttr" in SAFE else 4
        s4raw = const.tile([128, 6], F32)
        for j in range(nrange):
            nc.vector.reduce_sum(s4raw[:, j:j + 1],
                                 sums[:, nblk * j:nblk * (j + 1)],
                                 axis=mybir.AxisListType.X)
        s4 = const.tile([128, 4], F32)
        if "ttr" in SAFE:
            nc.vector.tensor_tensor(out=s4[:, 0:2], in0=s4raw[:, 0:2],
                                    in1=s4raw[:, 4:6], op=mybir.AluOpType.add)
            nc.vector.tensor_scalar_add(s4[:, 2:4], s4raw[:, 2:4], 0.0)
        else:
            nc.vector.tensor_scalar_add(s4[:], s4raw[:, 0:4], 0.0)
        nc.sync.dma_start(statin[:], s4[:])
        nc.gpsimd.collective_compute(
            "AllReduce", mybir.AluOpType.add, replica_groups=groups,
            ins=[statin.opt()], outs=[statout.opt()])
        s4g = const.tile([128, 4], F32)
        nc.sync.dma_start(s4g[:], statout[:])

        n_total = float(n_cores * rpc)
        mu = const.tile([128, 2], F32)
        nc.scalar.mul(mu[:], s4g[:, 0:2], 1.0 / n_total)
        ex2 = const.tile([128, 2], F32)
        nc.scalar.mul(ex2[:], s4g[:, 2:4], 1.0 / n_total)
        mu2 = const.tile([128, 2], F32)
        nc.vector.tensor_tensor(out=mu2[:], in0=mu[:], in1=mu[:],
                                op=mybir.AluOpType.mult)
        nmu2 = const.tile([128, 2], F32)
        nc.scalar.mul(nmu2[:], mu2[:], -1.0)
        var = const.tile([128, 2], F32)
        nc.vector.tensor_tensor(out=var[:], in0=ex2[:], in1=nmu2[:],
                                op=mybir.AluOpType.add)
        vare = const.tile([128, 2], F32)
        nc.vector.tensor_scalar_add(vare[:], var[:], EPS)
        std = const.tile([128, 2], F32)
        nc.scalar.activation(std[:], vare[:], mybir.ActivationFunctionType.Sqrt,
                             bias=0.0)
        rstd = const.tile([128, 2], F32)
        nc.vector.reciprocal(rstd[:], std[:])
        # a = gamma * rstd ; baff = beta - mu * a
        a_t = const.tile([128, 2], F32)
        nc.vector.tensor_tensor(out=a_t[:], in0=gb[:, 0:2], in1=rstd[:],
                                op=mybir.AluOpType.mult)
        mua = const.tile([128, 2], F32)
        nc.vector.tensor_tensor(out=mua[:], in0=mu[:], in1=a_t[:],
                                op=mybir.AluOpType.mult)
        nmua = const.tile([128, 2], F32)
        nc.scalar.mul(nmua[:], mua[:], -1.0)
        baff = const.tile([128, 2], F32)
        nc.vector.tensor_tensor(out=baff[:], in0=gb[:, 2:4], in1=nmua[:],
                                op=mybir.AluOpType.add)

        if "dbg" in SAFE:
            nzz = n_cores * W * 2 // 128
            dbt = const.tile([128, nzz * 128], F16, name="dbt")
            nc.sync.dma_start(dbt[:].rearrange("p (c f) -> p c f", c=nzz),
                              gslab[:].rearrange("s w (t f) -> (s w t) f", t=2)
                              .rearrange("(c p) f -> p c f", p=128))
            nc.sync.dma_start(d_dbg1.rearrange("(c p) f -> p c f", p=128),
                              dbt[:].rearrange("p (c f) -> p c f", c=nzz))
            nc.sync.dma_start(d_dbg2, hsb[0][:])
            db3 = const.tile([128, 12], F32, name="db3")
            nc.vector.tensor_tensor(out=db3[:, 0:4], in0=s4g[:], in1=s4g[:],
                                    op=mybir.AluOpType.bypass)
            nc.vector.tensor_tensor(out=db3[:, 4:6], in0=mu[:], in1=mu[:],
                                    op=mybir.AluOpType.bypass)
            nc.vector.tensor_tensor(out=db3[:, 6:8], in0=var[:], in1=var[:],
                                    op=mybir.AluOpType.bypass)
            nc.vector.tensor_tensor(out=db3[:, 8:10], in0=a_t[:], in1=a_t[:],
                                    op=mybir.AluOpType.bypass)
            nc.vector.tensor_tensor(out=db3[:, 10:12], in0=s4[:, 0:2],
                                    in1=s4[:, 0:2], op=mybir.AluOpType.bypass)
            nc.sync.dma_start(d_dbg3, db3[:])

        # ---- P3: out = relu(h * a + b), streamed out ----
        d_out_v = d_out.rearrange("(c p) n -> p c n", p=128)
        for b in range(nblk):
            osb = strm.tile([128, 2 * BLK], F32, name="osb", tag="osb", bufs=4)
            for m in range(2):
                nc.scalar.activation(osb[:, BLK * m:BLK * (m + 1)],
                                     hsb[m][:, BLK * b:BLK * (b + 1)],
                                     mybir.ActivationFunctionType.Relu,
                                     scale=a_t[:, m:m + 1], bias=baff[:, m:m + 1])
            eng = nc.sync if (b % 2 == 0 or "gpst" in SAFE) else nc.gpsimd
            eng.dma_start(
                d_out_v[:, :, BLK * b:BLK * (b + 1)],
                osb[:].rearrange("p (c n) -> p c n", c=2))

    nc.compile()
    return nc


def prep_core_inputs(x, atom_idx, r0, r1, slot, gstart, glast, inv_cnt, n):
    """Host-side shard prep for one core (unit metadata is global).
    gstart/glast: per-global-unit owned-range arrays."""
    rpc = r1 - r0
    nblk = rpc // BLK
    ngrp = nblk // GSZ
    u0 = r0 // (BLK * GSZ)          # first global unit of this core
    xs = x[r0:r1]

    # pre-tiled transposed x: [block, partition(=feat%128), chunk*512+row]
    xt = np.ascontiguousarray(
        xs.reshape(nblk, BLK, 3, 128).transpose(0, 3, 2, 1)
    ).reshape(nblk, 128, 3 * BLK)

    # x_ae row-major f16 tiles: nblk own blocks + 1 halo block (next core)
    XW = TPB * 2 * N_AE
    xae = np.zeros((nblk + 1, 128, XW), dtype=np.float16)
    xae[:nblk] = xs[:, :2 * N_AE].astype(np.float16).reshape(
        nblk, TPB, 128, 2 * N_AE).transpose(0, 2, 1, 3).reshape(nblk, 128, XW)
    if r1 < n:
        xae[nblk] = x[r1:r1 + BLK, :2 * N_AE].astype(np.float16).reshape(
            TPB, 128, 2 * N_AE).transpose(1, 0, 2).reshape(128, XW)

    # one-hot tiles per unit: [ngrp, 128, (GSZ+1)*TPB*slot]
    oh = np.zeros((ngrp, 128, (GSZ + 1) * TPB * slot), dtype=np.float16)
    scl = np.zeros((slot, ngrp), dtype=np.float32)
    lanes = np.arange(slot)
    for g in range(ngrp):
        u = u0 + g
        gs, gl = gstart[u], glast[u]
        scl[:, g] = inv_cnt[np.minimum(gs + lanes, NUM_SEG - 1)]
        ubase = r0 + BLK * GSZ * g
        for j in range(GSZ + 1):
            if j < GSZ:
                rows = atom_idx[ubase + BLK * j:ubase + BLK * (j + 1)]
                sel = rows >= gs                   # exclude prev unit's seg
            else:
                rr0 = ubase + BLK * GSZ
                rows = atom_idx[rr0:rr0 + BLK]
                if len(rows) == 0:
                    continue
                sel = rows == gl                   # halo: only our last seg
            lane = (rows - gs).astype(np.int64)
            for t in range(TPB):
                s = sel[128 * t:128 * (t + 1)]
                ln = lane[128 * t:128 * (t + 1)]
                p = np.flatnonzero(np.asarray(s))
                oh[g, p, (TPB * j + t) * slot + ln[p]] = 1.0

    return {"xt": xt.astype(np.float16), "xae": xae, "oh": oh, "scl": scl}


def prep(x, atom_idx, ele_idx, W1, b1, gamma, beta, n_cores=8, build=True):
    x = np.asarray(x, dtype=np.float32)
    atom_idx = np.asarray(atom_idx).astype(np.int64)
    ele_idx = np.asarray(ele_idx).astype(np.int64)
    W1 = np.asarray(W1, dtype=np.float32)
    gamma = np.asarray(gamma, dtype=np.float32)
    beta = np.asarray(beta, dtype=np.float32)

    n = x.shape[0]
    assert n % n_cores == 0
    rpc = n // n_cores
    assert rpc % (BLK * GSZ) == 0
    nblk = rpc // BLK
    ngrp = nblk // GSZ
    nunit = n_cores * ngrp
    urows = BLK * GSZ
    assert np.all(np.diff(atom_idx) >= 0), "atom_idx must be sorted"

    counts = np.bincount(atom_idx, minlength=NUM_SEG).astype(np.int64)
    inv_cnt = (1.0 / np.maximum(counts, 1)).astype(np.float32)

    # global unit ownership: unit u owns segs [gstart[u], glast[u]]
    ufirst = atom_idx[np.arange(nunit) * urows]            # seg of first row
    ulast = atom_idx[np.arange(1, nunit + 1) * urows - 1]  # seg of last row
    gstart = np.empty(nunit, dtype=np.int64)
    gstart[0] = 0
    for u in range(1, nunit):
        gstart[u] = max(ufirst[u], ulast[u - 1] + 1)
    spans = ulast - gstart + 1
    assert spans.min() >= 1, "a unit owns no segment"
    slot = int(((spans.max() + 2) + 7) // 8 * 8)
    assert slot <= 128, f"unit owned span too large: {spans.max()}"
    # halo containment: a unit's last seg may spill only into the next
    # unit's first 512 rows
    for u in range(nunit - 1):
        r = urows * (u + 1) + BLK
        if r < n:
            assert atom_idx[r] > ulast[u], "segment spans past halo window"

    # global seg -> merged slab row (2*row for atom half, 2*row+1 for ele)
    W = ngrp * slot
    owner = np.full(NUM_SEG, -1, dtype=np.int64)
    for u in range(nunit - 1, -1, -1):
        owner[gstart[u]:ulast[u] + 1] = u
    slabrow = np.full(NUM_SEG, slot - 1, dtype=np.int64)   # empty segs -> zero row
    m = owner >= 0
    su = owner[m]
    slabrow[m] = (su // ngrp) * W + (su % ngrp) * slot + \
        (np.arange(NUM_SEG)[m] - gstart[su])
    arow = 2 * slabrow[atom_idx]
    erow = 2 * slabrow[ele_idx] + 1

    in_maps = []
    for c in range(n_cores):
        r0 = rpc * c
        im = prep_core_inputs(x, atom_idx, r0, r0 + rpc, slot, gstart, glast=ulast,
                              inv_cnt=inv_cnt, n=n)
        gidx = np.zeros((nblk, 128, 2 * (BLK // 16)), dtype=np.int16)
        for b in range(nblk):
            merged = np.concatenate([arow[r0 + BLK * b:r0 + BLK * (b + 1)],
                                     erow[r0 + BLK * b:r0 + BLK * (b + 1)]])
            gidx[b] = _wrap_idx16(merged)
        im["gidx"] = gidx
        if "mg" in SAFE:
            ga = np.zeros((nblk, 128, BLK // 16), dtype=np.int16)
            ge = np.zeros((nblk, 128, BLK // 16), dtype=np.int16)
            for b in range(nblk):
                ga[b] = _wrap_idx16(slabrow[atom_idx[r0 + BLK * b:r0 + BLK * (b + 1)]])
                ge[b] = _wrap_idx16(slabrow[ele_idx[r0 + BLK * b:r0 + BLK * (b + 1)]])
            im["gidxa"] = ga
            im["gidxe"] = ge
        im["wx"] = np.ascontiguousarray(
            np.concatenate([W1[0:128], W1[256:384], W1[512:640]], axis=0)
        ).astype(np.float16)
        im["wpa"] = W1[128:256].astype(np.float16)
        im["wpe"] = W1[384:512].astype(np.float16)
        gbt = np.zeros((128, 4), dtype=np.float32)
        gbt[:, 0:2] = gamma.reshape(2, 128).T
        gbt[:, 2:4] = beta.reshape(2, 128).T
        im["gb"] = gbt
        in_maps.append(im)

    nc = build_program(n_cores, rpc, slot) if build else None
    global LAST_NC
    LAST_NC = nc
    return nc, in_maps


def run(x, atom_idx, ele_idx, W1, b1, gamma, beta, n_cores=8, runner=None):
    nc, in_maps = prep(x, atom_idx, ele_idx, W1, b1, gamma, beta, n_cores)
    if runner is None:
        res = run_bass_kernel_spmd(nc, in_maps, core_ids=list(range(n_cores)))
        outs = [res.results[c]["out"] for c in range(n_cores)]
    else:
        outs = runner(nc, in_maps)

    full = np.concatenate(outs, axis=1)          # [256, n]
    return np.ascontiguousarray(full.T)          # [n, 256]


def kernel(**inputs):
    return run(inputs["x"], inputs["atom_idx"], inputs["ele_idx"],
               inputs["W1"], inputs["b1"], inputs["gamma"], inputs["beta"])


# revision 22
# speedup vs baseline: 1.1337x; 1.0395x over previous
"""Trainium2 Bass kernel for nn_DistLayer (segment-mean pooling + fc + BatchNorm + ReLU).

Contract: kernel(**inputs) takes FULL unsharded numpy inputs and returns the
FULL [131072, 256] float32 output. Internally shards rows across 8 NeuronCores.

Math (reference):
    pooled_atom = segment_mean(x[:, :128], atom_idx)[atom_idx]
    pooled_ele  = segment_mean(x[:, 128:256], atom_idx)[ele_idx]
    h = concat([x_atom, pooled_atom, x_ele, pooled_ele, x_dist]) @ W1 + b1
    out = relu(batchnorm(h))                    (training-mode batch stats)

v3 structure (per core, h kept feature-major "h^T" [256, rows]):
  P1   : the N rows are cut into 64 global "units" of 2048 rows (8/core).
         Each unit OWNS a disjoint contiguous segment range (earliest unit
         containing a segment owns it; a unit's rows belonging to the previous
         unit's last segment are excluded from its one-hot, and a 5th "halo"
         tile -- the next unit's first 512 rows -- completes the last owned
         segment). One-hot matmuls produce COMPLETE global segment sums per
         owned lane, scaled by 1/count and flushed (f16) DENSELY into the
         core's slab: no scatter, no table zeroing, no cast pass.
  AG   : AllGather the 8 per-core slabs -> gslab [8, ngrp*SLOT, 256] f16.
         Gather indices are host-remapped to (core, unit, lane) slots. Issued
         early; overlaps phase A.
  A    : h_x^T = Wx^T x^T per block, flushed to persistent SBUF hsb.
  B    : one merged dma_gather per block pulls both pooled_atom and pooled_ele
         rows from gslab; two f16 matmuls per 128-feature chunk; DVE
         tensor_tensor_reduce adds PSUM into hsb and emits sum(h) per block;
         an Act Square pass emits sum(h^2).
  stats: AllGather per-core [128,4] sums, reduce locally, build scale/bias.
  P3   : fused scale+bias+relu pass streamed to DRAM out.
"""

import os
from contextlib import ExitStack

import numpy as np

import concourse.bass as bass
import concourse.tile as tile
from concourse import bacc, mybir
from concourse.bass_utils import run_bass_kernel_spmd

LAST_NC = None  # most recent built program (for cost-model timing in test.py)
SAFE = set(filter(None, os.environ.get("K_SAFE", "").split(","))) | {"mg"}

F32 = mybir.dt.float32
F32R = mybir.dt.float32r
F16 = mybir.dt.float16
I16 = mybir.dt.int16
I32 = mybir.dt.int32

N_AE = 128
NUM_SEG = 4096
EPS = 1e-5
D_IN = 384            # x feature dim
D_OUT = 256           # output feature dim
BLK = 512             # rows per block
TPB = BLK // 128      # row-tiles per block
GSZ = 4               # blocks per unit (2048 rows)


def _wrap_idx16(idx):
    """dma_gather index layout: idx i at [i%16, i//16], replicated to 128 partitions."""
    n = idx.shape[0]
    w = idx.reshape(n // 16, 16).T.astype(np.int16)   # [16, n/16]
    return np.tile(w, (8, 1))                          # [128, n/16]


def build_program(n_cores, rpc, slot):
    """Build the (core-uniform) bass program. rpc = rows per core."""
    nblk = rpc // BLK
    ngrp = nblk // GSZ
    W = ngrp * slot       # slab rows per core
    nc = bacc.Bacc("TRN2", target_bir_lowering=False, debug=False,
                   num_devices=n_cores)

    # ---- I/O tensors (per-core) ----
    d_xt = nc.dram_tensor("xt", [nblk, 128, 3 * BLK], F16, kind="ExternalInput").ap()
    d_xae = nc.dram_tensor("xae", [nblk + 1, 128, TPB * 2 * N_AE], F16, kind="ExternalInput").ap()
    d_oh = nc.dram_tensor("oh", [ngrp, 128, (GSZ + 1) * TPB * slot], F16, kind="ExternalInput").ap()
    d_scl = nc.dram_tensor("scl", [slot, ngrp], F32, kind="ExternalInput").ap()
    d_gidx = nc.dram_tensor("gidx", [nblk, 128, 2 * (BLK // 16)], I16, kind="ExternalInput").ap()
    if "mg" in SAFE:
        d_gidxa = nc.dram_tensor("gidxa", [nblk, 128, BLK // 16], I16, kind="ExternalInput").ap()
        d_gidxe = nc.dram_tensor("gidxe", [nblk, 128, BLK // 16], I16, kind="ExternalInput").ap()
    d_wx = nc.dram_tensor("wx", [D_IN, D_OUT], F16, kind="ExternalInput").ap()
    d_wpa = nc.dram_tensor("wpa", [N_AE, D_OUT], F16, kind="ExternalInput").ap()
    d_wpe = nc.dram_tensor("wpe", [N_AE, D_OUT], F16, kind="ExternalInput").ap()
    d_gb = nc.dram_tensor("gb", [128, 4], F32, kind="ExternalInput").ap()

    d_out = nc.dram_tensor("out", [D_OUT, rpc], F32, kind="ExternalOutput").ap()
    if "dbg" in SAFE:
        d_dbg1 = nc.dram_tensor("dbg1", [n_cores * W * 2, 128], F16, kind="ExternalOutput").ap()
        d_dbg2 = nc.dram_tensor("dbg2", [128, rpc], F32, kind="ExternalOutput").ap()
        d_dbg3 = nc.dram_tensor("dbg3", [128, 12], F32, kind="ExternalOutput").ap()

    groups = [list(range(n_cores))]

    with tile.TileContext(nc) as tc, ExitStack() as ctx:
        const = ctx.enter_context(tc.tile_pool(name="const", bufs=1))
        store = ctx.enter_context(tc.tile_pool(name="store", bufs=1))
        strm = ctx.enter_context(tc.tile_pool(name="strm", bufs=3))
        ps = ctx.enter_context(tc.tile_pool(name="ps", bufs=2, space="PSUM"))
        dram = ctx.enter_context(tc.tile_pool(name="dram", bufs=1, space="DRAM"))

        # internal DRAM
        pslab = dram.tile([W, D_OUT], F16)                # AG input slab
        gslab = dram.tile([n_cores, W, D_OUT], F16, addr_space="Shared")
        statin = dram.tile([128, 4], F32)
        statout = dram.tile([n_cores, 128, 4], F32, addr_space="Shared")

        # ---- constants in SBUF ----
        wxr = const.tile([128, 3 * D_OUT], F16)
        nc.gpsimd.dma_start(wxr[:].rearrange("p (c f) -> p c f", c=3),
                            d_wx.rearrange("(c p) f -> p c f", p=128))
        wpa = const.tile([128, D_OUT], F16)
        nc.scalar.dma_start(wpa[:], d_wpa[:])
        wpe = const.tile([128, D_OUT], F16)
        nc.scalar.dma_start(wpe[:], d_wpe[:])
        scl = const.tile([slot, ngrp], F32)
        nc.scalar.dma_start(scl[:], d_scl[:])
        gb = const.tile([128, 4], F32)
        nc.scalar.dma_start(gb[:], d_gb[:])
        gsb = const.tile([128, nblk * 2 * (BLK // 16)], I16)
        nc.scalar.dma_start(gsb[:].rearrange("p (b w) -> p b w", b=nblk),
                            d_gidx.rearrange("b p w -> p b w"))
        if "mg" in SAFE:
            gsba = const.tile([128, nblk * (BLK // 16)], I16)
            nc.scalar.dma_start(gsba[:].rearrange("p (b w) -> p b w", b=nblk),
                                d_gidxa.rearrange("b p w -> p b w"))
            gsbe = const.tile([128, nblk * (BLK // 16)], I16)
            nc.scalar.dma_start(gsbe[:].rearrange("p (b w) -> p b w", b=nblk),
                                d_gidxe.rearrange("b p w -> p b w"))

        # persistent h^T store: 2 chunks of [128, rpc]
        hsb = [store.tile([128, rpc], F32, name=f"hsb{m}", tag=f"hsb{m}")
               for m in range(2)]
        sums = store.tile([128, 6 * nblk], F32)   # [shq0|shq1|shh0|shh1|sa0|sa1]

        # ---- P1: per-unit one-hot matmuls -> dense f16 slab flush ----
        XW = TPB * 2 * N_AE
        xtiles = []

        def load_xae(b):
            t = strm.tile([128, XW], F16, name="xae", tag="xae", bufs=9)
            nc.sync.dma_start(t[:], d_xae[b])
            xtiles.append(t)

        for b in range(GSZ):
            load_xae(b)
        for g in range(ngrp):
            for b in range(GSZ * (g + 1), min(GSZ * (g + 2), nblk + 1)):
                load_xae(b)
            ohx = strm.tile([128, (GSZ + 1) * TPB * slot], F16,
                            name="ohx", tag="ohx", bufs=2)
            nc.sync.dma_start(ohx[:], d_oh[g])
            seg = ps.tile([slot, D_OUT], F32, name="seg", tag="seg")
            for j in range(GSZ + 1):
                xt_b = xtiles[GSZ * g + j]
                for t in range(TPB):
                    nc.tensor.matmul(seg[:],
                                     ohx[:, (TPB * j + t) * slot:
                                         (TPB * j + t + 1) * slot],
                                     xt_b[:, 2 * N_AE * t:2 * N_AE * (t + 1)],
                                     start=(j == 0 and t == 0),
                                     stop=(j == GSZ and t == TPB - 1))
            ssb = strm.tile([slot, D_OUT], F16, name="ssb", tag="ssb", bufs=2)
            # scale by 1/global_count while flushing PSUM -> SBUF (to f16)
            nc.scalar.activation(ssb[:], seg[:],
                                 mybir.ActivationFunctionType.Identity,
                                 bias=0.0, scale=scl[:, g:g + 1])
            nc.scalar.dma_start(pslab[slot * g:slot * (g + 1)], ssb[:])

        # ---- AllGather the slabs ----
        nc.gpsimd.collective_compute(
            "AllGather", mybir.AluOpType.bypass, replica_groups=groups,
            ins=[pslab.opt()], outs=[gslab.opt()])

        # ---- phase A: h_x^T = Wx^T x^T per block -> hsb ----
        # hold the x loads out of the scheduler's P1 window so the slab
        # AllGather (which gates phase B) is issued as early as possible
        for b in range(nblk):
            xtr = strm.tile([128, 3 * BLK], F16, name="xtr", tag="xtr")
            with tc.tile_wait_until(0.030, enable="wait" not in SAFE):
                nc.sync.dma_start(xtr[:], d_xt[b])
            for m in range(2):
                hp = ps.tile([128, BLK], F32, name=f"hp{m}", tag=f"hp{m}")
                for k in range(3):
                    nc.tensor.matmul(hp[:],
                                     wxr[:, D_OUT * k + 128 * m:
                                         D_OUT * k + 128 * (m + 1)],
                                     xtr[:, BLK * k:BLK * (k + 1)],
                                     start=(k == 0), stop=(k == 2))
                if "ttr" in SAFE:
                    nc.scalar.activation(
                        hsb[m][:, BLK * b:BLK * (b + 1)], hp[:],
                        mybir.ActivationFunctionType.Copy,
                        accum_out=sums[:, 4 * nblk + nblk * m + b:
                                       4 * nblk + nblk * m + b + 1])
                else:
                    nc.scalar.copy(hsb[m][:, BLK * b:BLK * (b + 1)], hp[:])

        # ---- phase B: merged gather + pooled matmuls, add into hsb ----
        gview = gslab[:].rearrange("s w (t f) -> (s w t) f", t=2)
        gview2 = gslab[:].rearrange("s w f -> (s w) f")
        for b in range(nblk):
            gat = strm.tile([128, 2 * BLK], F16, name="gat", tag="gat", bufs=4)
            if "mg" in SAFE:
                nc.gpsimd.dma_gather(
                    out_ap=gat[:, 0:BLK].rearrange("p (a n) -> p a n", a=1),
                    in_ap=gview2[:, 0:N_AE],
                    idxs_ap=gsba[:, (BLK // 16) * b:(BLK // 16) * (b + 1)],
                    num_idxs=BLK, num_idxs_reg=BLK,
                    elem_size=N_AE, elem_step=D_OUT, transpose=True)
                nc.gpsimd.dma_gather(
                    out_ap=gat[:, BLK:2 * BLK].rearrange("p (a n) -> p a n", a=1),
                    in_ap=gview2[:, N_AE:2 * N_AE],
                    idxs_ap=gsbe[:, (BLK // 16) * b:(BLK // 16) * (b + 1)],
                    num_idxs=BLK, num_idxs_reg=BLK,
                    elem_size=N_AE, elem_step=D_OUT, transpose=True)
            else:
                nc.gpsimd.dma_gather(
                    out_ap=gat[:].rearrange("p (a n) -> p a n", a=1),
                    in_ap=gview,
                    idxs_ap=gsb[:, 2 * (BLK // 16) * b:2 * (BLK // 16) * (b + 1)],
                    num_idxs=2 * BLK, num_idxs_reg=2 * BLK,
                    elem_size=N_AE, elem_step=N_AE, transpose=True)
            for m in range(2):
                hq = ps.tile([128, BLK], F32, name=f"hp{m}", tag=f"hp{m}")
                nc.tensor.matmul(hq[:], wpa[:, 128 * m:128 * (m + 1)],
                                 gat[:, 0:BLK], start=True, stop=False)
                nc.tensor.matmul(hq[:], wpe[:, 128 * m:128 * (m + 1)],
                                 gat[:, BLK:2 * BLK], start=False, stop=True)
                # hsb += pooled part; accumulate sum(h) on the fly
                if "ttr" in SAFE:
                    hqs = strm.tile([128, BLK], F32, name="hqs", tag="hqs", bufs=3)
                    nc.scalar.activation(
                        hqs[:], hq[:],
                        mybir.ActivationFunctionType.Copy,
                        accum_out=sums[:, nblk * m + b:nblk * m + b + 1])
                    nc.vector.tensor_tensor(
                        out=hsb[m][:, BLK * b:BLK * (b + 1)],
                        in0=hsb[m][:, BLK * b:BLK * (b + 1)],
                        in1=hqs[:], op=mybir.AluOpType.add)
                else:
                    nc.vector.scalar_tensor_tensor(
                        out=hsb[m][:, BLK * b:BLK * (b + 1)],
                        in0=hsb[m][:, BLK * b:BLK * (b + 1)],
                        scalar=0.0, in1=hq[:],
                        op0=mybir.AluOpType.add, op1=mybir.AluOpType.add,
                        accum_out=sums[:, nblk * m + b:nblk * m + b + 1])
                dump = strm.tile([128, BLK], F32, name="dump", tag="dump", bufs=1)
                nc.scalar.activation(
                    dump[:], hsb[m][:, BLK * b:BLK * (b + 1)],
                    mybir.ActivationFunctionType.Square,
                    accum_out=sums[:, 2 * nblk + nblk * m + b:
                                   2 * nblk + nblk * m + b + 1])

        # ---- BN stats: reduce blocks, AllGather cores, build scale/bias ----
        nrange = 6 if "ttr" in SAFE else 4
        s4raw = const.tile([128, 6], F32)
        for j in range(nrange):
            nc.vector.reduce_sum(s4raw[:, j:j + 1],
                                 sums[:, nblk * j:nblk * (j + 1)],
                                 axis=mybir.AxisListType.X)
        s4 = const.tile([128, 4], F32)
        if "ttr" in SAFE:
            nc.vector.tensor_tensor(out=s4[:, 0:2], in0=s4raw[:, 0:2],
                                    in1=s4raw[:, 4:6], op=mybir.AluOpType.add)
            nc.vector.tensor_scalar_add(s4[:, 2:4], s4raw[:, 2:4], 0.0)
        else:
            nc.vector.tensor_scalar_add(s4[:], s4raw[:, 0:4], 0.0)
        nc.sync.dma_start(statin[:], s4[:])
        nc.gpsimd.collective_compute(
            "AllGather", mybir.AluOpType.bypass, replica_groups=groups,
            ins=[statin.opt()], outs=[statout.opt()])
        s4a = const.tile([128, 4 * n_cores], F32)
        nc.sync.dma_start(s4a[:].rearrange("p (s f) -> s p f", s=n_cores),
                          statout[:])
        su = const.tile([128, 16], F32)
        for j in range(4):
            nc.vector.tensor_tensor(out=su[:, 4 * j:4 * (j + 1)],
                                    in0=s4a[:, 8 * j:8 * j + 4],
                                    in1=s4a[:, 8 * j + 4:8 * j + 8],
                                    op=mybir.AluOpType.add)
        sv = const.tile([128, 8], F32)
        for j in range(2):
            nc.vector.tensor_tensor(out=sv[:, 4 * j:4 * (j + 1)],
                                    in0=su[:, 8 * j:8 * j + 4],
                                    in1=su[:, 8 * j + 4:8 * j + 8],
                                    op=mybir.AluOpType.add)
        s4g = const.tile([128, 4], F32)
        nc.vector.tensor_tensor(out=s4g[:], in0=sv[:, 0:4], in1=sv[:, 4:8],
                                op=mybir.AluOpType.add)

        n_total = float(n_cores * rpc)
        mu = const.tile([128, 2], F32)
        nc.scalar.mul(mu[:], s4g[:, 0:2], 1.0 / n_total)
        ex2 = const.tile([128, 2], F32)
        nc.scalar.mul(ex2[:], s4g[:, 2:4], 1.0 / n_total)
        mu2 = const.tile([128, 2], F32)
        nc.vector.tensor_tensor(out=mu2[:], in0=mu[:], in1=mu[:],
                                op=mybir.AluOpType.mult)
        nmu2 = const.tile([128, 2], F32)
        nc.scalar.mul(nmu2[:], mu2[:], -1.0)
        var = const.tile([128, 2], F32)
        nc.vector.tensor_tensor(out=var[:], in0=ex2[:], in1=nmu2[:],
                                op=mybir.AluOpType.add)
        vare = const.tile([128, 2], F32)
        nc.vector.tensor_scalar_add(vare[:], var[:], EPS)
        std = const.tile([128, 2], F32)
        nc.scalar.activation(std[:], vare[:], mybir.ActivationFunctionType.Sqrt,
                             bias=0.0)
        rstd = const.tile([128, 2], F32)
        nc.vector.reciprocal(rstd[:], std[:])
        # a = gamma * rstd ; baff = beta - mu * a
        a_t = const.tile([128, 2], F32)
        nc.vector.tensor_tensor(out=a_t[:], in0=gb[:, 0:2], in1=rstd[:],
                                op=mybir.AluOpType.mult)
        mua = const.tile([128, 2], F32)
        nc.vector.tensor_tensor(out=mua[:], in0=mu[:], in1=a_t[:],
                                op=mybir.AluOpType.mult)
        nmua = const.tile([128, 2], F32)
        nc.scalar.mul(nmua[:], mua[:], -1.0)
        baff = const.tile([128, 2], F32)
        nc.vector.tensor_tensor(out=baff[:], in0=gb[:, 2:4], in1=nmua[:],
                                op=mybir.AluOpType.add)

        if "dbg" in SAFE:
            nzz = n_cores * W * 2 // 128
            dbt = const.tile([128, nzz * 128], F16, name="dbt")
            nc.sync.dma_start(dbt[:].rearrange("p (c f) -> p c f", c=nzz),
                              gslab[:].rearrange("s w (t f) -> (s w t) f", t=2)
                              .rearrange("(c p) f -> p c f", p=128))
            nc.sync.dma_start(d_dbg1.rearrange("(c p) f -> p c f", p=128),
                              dbt[:].rearrange("p (c f) -> p c f", c=nzz))
            nc.sync.dma_start(d_dbg2, hsb[0][:])
            db3 = const.tile([128, 12], F32, name="db3")
            nc.vector.tensor_tensor(out=db3[:, 0:4], in0=s4g[:], in1=s4g[:],
                                    op=mybir.AluOpType.bypass)
            nc.vector.tensor_tensor(out=db3[:, 4:6], in0=mu[:], in1=mu[:],
                                    op=mybir.AluOpType.bypass)
            nc.vector.tensor_tensor(out=db3[:, 6:8], in0=var[:], in1=var[:],
                                    op=mybir.AluOpType.bypass)
            nc.vector.tensor_tensor(out=db3[:, 8:10], in0=a_t[:], in1=a_t[:],
                                    op=mybir.AluOpType.bypass)
            nc.vector.tensor_tensor(out=db3[:, 10:12], in0=s4[:, 0:2],
                                    in1=s4[:, 0:2], op=mybir.AluOpType.bypass)
            nc.sync.dma_start(d_dbg3, db3[:])

        # ---- P3: out = relu(h * a + b), streamed out ----
        d_out_v = d_out.rearrange("(c p) n -> p c n", p=128)
        for b in range(nblk):
            osb = strm.tile([128, 2 * BLK], F32, name="osb", tag="osb", bufs=4)
            for m in range(2):
                nc.scalar.activation(osb[:, BLK * m:BLK * (m + 1)],
                                     hsb[m][:, BLK * b:BLK * (b + 1)],
                                     mybir.ActivationFunctionType.Relu,
                                     scale=a_t[:, m:m + 1], bias=baff[:, m:m + 1])
            eng = nc.sync if (b % 2 == 0 or "gpst" in SAFE) else nc.gpsimd
            eng.dma_start(
                d_out_v[:, :, BLK * b:BLK * (b + 1)],
                osb[:].rearrange("p (c n) -> p c n", c=2))

    nc.compile()
    return nc


def prep_core_inputs(x, atom_idx, r0, r1, slot, gstart, glast, inv_cnt, n):
    """Host-side shard prep for one core (unit metadata is global).
    gstart/glast: per-global-unit owned-range arrays."""
    rpc = r1 - r0
    nblk = rpc // BLK
    ngrp = nblk // GSZ
    u0 = r0 // (BLK * GSZ)          # first global unit of this core
    xs = x[r0:r1]

    # pre-tiled transposed x: [block, partition(=feat%128), chunk*512+row]
    xt = np.ascontiguousarray(
        xs.reshape(nblk, BLK, 3, 128).transpose(0, 3, 2, 1)
    ).reshape(nblk, 128, 3 * BLK)

    # x_ae row-major f16 tiles: nblk own blocks + 1 halo block (next core)
    XW = TPB * 2 * N_AE
    xae = np.zeros((nblk + 1, 128, XW), dtype=np.float16)
    xae[:nblk] = xs[:, :2 * N_AE].astype(np.float16).reshape(
        nblk, TPB, 128, 2 * N_AE).transpose(0, 2, 1, 3).reshape(nblk, 128, XW)
    if r1 < n:
        xae[nblk] = x[r1:r1 + BLK, :2 * N_AE].astype(np.float16).reshape(
            TPB, 128, 2 * N_AE).transpose(1, 0, 2).reshape(128, XW)

    # one-hot tiles per unit: [ngrp, 128, (GSZ+1)*TPB*slot]
    oh = np.zeros((ngrp, 128, (GSZ + 1) * TPB * slot), dtype=np.float16)
    scl = np.zeros((slot, ngrp), dtype=np.float32)
    lanes = np.arange(slot)
    for g in range(ngrp):
        u = u0 + g
        gs, gl = gstart[u], glast[u]
        scl[:, g] = inv_cnt[np.minimum(gs + lanes, NUM_SEG - 1)]
        ubase = r0 + BLK * GSZ * g
        for j in range(GSZ + 1):
            if j < GSZ:
                rows = atom_idx[ubase + BLK * j:ubase + BLK * (j + 1)]
                sel = rows >= gs                   # exclude prev unit's seg
            else:
                rr0 = ubase + BLK * GSZ
                rows = atom_idx[rr0:rr0 + BLK]
                if len(rows) == 0:
                    continue
                sel = rows == gl                   # halo: only our last seg
            lane = (rows - gs).astype(np.int64)
            for t in range(TPB):
                s = sel[128 * t:128 * (t + 1)]
                ln = lane[128 * t:128 * (t + 1)]
                p = np.flatnonzero(np.asarray(s))
                oh[g, p, (TPB * j + t) * slot + ln[p]] = 1.0

    return {"xt": xt.astype(np.float16), "xae": xae, "oh": oh, "scl": scl}


def prep(x, atom_idx, ele_idx, W1, b1, gamma, beta, n_cores=8, build=True):
    x = np.asarray(x, dtype=np.float32)
    atom_idx = np.asarray(atom_idx).astype(np.int64)
    ele_idx = np.asarray(ele_idx).astype(np.int64)
    W1 = np.asarray(W1, dtype=np.float32)
    gamma = np.asarray(gamma, dtype=np.float32)
    beta = np.asarray(beta, dtype=np.float32)

    n = x.shape[0]
    assert n % n_cores == 0
    rpc = n // n_cores
    assert rpc % (BLK * GSZ) == 0
    nblk = rpc // BLK
    ngrp = nblk // GSZ
    nunit = n_cores * ngrp
    urows = BLK * GSZ
    assert np.all(np.diff(atom_idx) >= 0), "atom_idx must be sorted"

    counts = np.bincount(atom_idx, minlength=NUM_SEG).astype(np.int64)
    inv_cnt = (1.0 / np.maximum(counts, 1)).astype(np.float32)

    # global unit ownership: unit u owns segs [gstart[u], glast[u]]
    ufirst = atom_idx[np.arange(nunit) * urows]            # seg of first row
    ulast = atom_idx[np.arange(1, nunit + 1) * urows - 1]  # seg of last row
    gstart = np.empty(nunit, dtype=np.int64)
    gstart[0] = 0
    for u in range(1, nunit):
        gstart[u] = max(ufirst[u], ulast[u - 1] + 1)
    spans = ulast - gstart + 1
    assert spans.min() >= 1, "a unit owns no segment"
    slot = int(((spans.max() + 2) + 7) // 8 * 8)
    assert slot <= 128, f"unit owned span too large: {spans.max()}"
    # halo containment: a unit's last seg may spill only into the next
    # unit's first 512 rows
    for u in range(nunit - 1):
        r = urows * (u + 1) + BLK
        if r < n:
            assert atom_idx[r] > ulast[u], "segment spans past halo window"

    # global seg -> merged slab row (2*row for atom half, 2*row+1 for ele)
    W = ngrp * slot
    owner = np.full(NUM_SEG, -1, dtype=np.int64)
    for u in range(nunit - 1, -1, -1):
        owner[gstart[u]:ulast[u] + 1] = u
    slabrow = np.full(NUM_SEG, slot - 1, dtype=np.int64)   # empty segs -> zero row
    m = owner >= 0
    su = owner[m]
    slabrow[m] = (su // ngrp) * W + (su % ngrp) * slot + \
        (np.arange(NUM_SEG)[m] - gstart[su])
    arow = 2 * slabrow[atom_idx]
    erow = 2 * slabrow[ele_idx] + 1

    in_maps = []
    for c in range(n_cores):
        r0 = rpc * c
        im = prep_core_inputs(x, atom_idx, r0, r0 + rpc, slot, gstart, glast=ulast,
                              inv_cnt=inv_cnt, n=n)
        gidx = np.zeros((nblk, 128, 2 * (BLK // 16)), dtype=np.int16)
        for b in range(nblk):
            merged = np.concatenate([arow[r0 + BLK * b:r0 + BLK * (b + 1)],
                                     erow[r0 + BLK * b:r0 + BLK * (b + 1)]])
            gidx[b] = _wrap_idx16(merged)
        im["gidx"] = gidx
        if "mg" in SAFE:
            ga = np.zeros((nblk, 128, BLK // 16), dtype=np.int16)
            ge = np.zeros((nblk, 128, BLK // 16), dtype=np.int16)
            for b in range(nblk):
                ga[b] = _wrap_idx16(slabrow[atom_idx[r0 + BLK * b:r0 + BLK * (b + 1)]])
                ge[b] = _wrap_idx16(slabrow[ele_idx[r0 + BLK * b:r0 + BLK * (b + 1)]])
            im["gidxa"] = ga
            im["gidxe"] = ge
        im["wx"] = np.ascontiguousarray(
            np.concatenate([W1[0:128], W1[256:384], W1[512:640]], axis=0)
        ).astype(np.float16)
        im["wpa"] = W1[128:256].astype(np.float16)
        im["wpe"] = W1[384:512].astype(np.float16)
        gbt = np.zeros((128, 4), dtype=np.float32)
        gbt[:, 0:2] = gamma.reshape(2, 128).T
        gbt[:, 2:4] = beta.reshape(2, 128).T
        im["gb"] = gbt
        in_maps.append(im)

    nc = build_program(n_cores, rpc, slot) if build else None
    global LAST_NC
    LAST_NC = nc
    return nc, in_maps


def run(x, atom_idx, ele_idx, W1, b1, gamma, beta, n_cores=8, runner=None):
    nc, in_maps = prep(x, atom_idx, ele_idx, W1, b1, gamma, beta, n_cores)
    if runner is None:
        res = run_bass_kernel_spmd(nc, in_maps, core_ids=list(range(n_cores)))
        outs = [res.results[c]["out"] for c in range(n_cores)]
    else:
        outs = runner(nc, in_maps)

    full = np.concatenate(outs, axis=1)          # [256, n]
    return np.ascontiguousarray(full.T)          # [n, 256]


def kernel(**inputs):
    return run(inputs["x"], inputs["atom_idx"], inputs["ele_idx"],
               inputs["W1"], inputs["b1"], inputs["gamma"], inputs["beta"])
